# revision 46
# baseline (speedup 1.0000x reference)
"""Multi-head self-attention (B=4, S=2048, D=1024, H=8) on 8 TRN2 NeuronCores.

Sharding: core c -> batch b=c//2, head-group g=c%2 (4 heads/core).
Each core computes its 4 heads' attention output [512, 2048] (transposed,
head-major, f16); the host gathers/reassembles the full [B, S, D] output.

Notes on the math: the reference adds the source mask per-QUERY (constant
along the key axis) before a softmax over keys, so the mask cancels exactly;
encoder_output_embedding and the target mask are unused by the reference.
The kernel therefore computes pure softmax(q k^T / sqrt(dh)) v.

Schedule (what got it from ~246us to ~228us):
- Input DMAs batched into few descriptors split across the two HWDGE
  engines (SP + ACT): dma_start issue is ~650ns serialized per engine;
  56 small DMAs made the PE idle ~11us at the start. A few dummy
  matmuls warm the PE p-state while the first tiles land.
- The 1/sqrt(dh) scale is folded into the exp activation's scale.
- Softmax denominators: eager DVE folds of the exp tiles; ones-matmul
  partition-reduce (8 replicated rows so the PSUM->SBUF copy runs 8
  lanes wide); [128,8] repartition DMA; DVE reciprocal (f16);
  departition DMA; broadcast back as [128,1024] (DRAM-bounce DMA for
  pipelined blocks, PE ones-matmul broadcast for the final chains).
- PV PSUM is copied to SBUF as soon as it stops so the bank frees
  without waiting on the reciprocal chain; the final multiply runs on
  GpSimd (both operands SBUF) to avoid DVE head-of-line blocking.
- Head 3's V projection is deferred out of the V phase and dripped into
  the last head's "superblock" (its two query blocks interleaved),
  which is otherwise exp(ACT)-bound; the superblock's first qk pair is
  emitted inside the previous block's tail to keep ACT streaming.
- Heads 1..3 project inside the previous head's attention loop
  (2 matmuls dripped per key step).
- Output is written f16 (half the DMA bytes; rel err stays ~6e-4).
"""
import math
from contextlib import ExitStack

import numpy as np

import concourse.bacc as bacc
import concourse.tile as tile
from concourse import mybir
from concourse.bass_utils import run_bass_kernel_spmd

N_CORES = 8
B, S, D, H = 4, 2048, 1024, 8
DH = 128                    # head dim
HPC = 4                     # heads per core
DHG = HPC * DH              # 512: projected width per core
SCALE = 1.0 / math.sqrt(DH)

F32 = mybir.dt.float32
F16 = mybir.dt.float16

TRACE = False               # test.py flips this for profiling runs
_CACHE = {}

KT = S // 128               # 16 key tiles
ND = D // 128               # 8 contraction tiles
NSB = S // 512              # 4 column blocks of x


def _emit(tc, nc, xt_ap, wq_ap, wk_ap, wv_ap, out_ap):
    with ExitStack() as ctx:
        p_x = ctx.enter_context(tc.tile_pool(name="x", bufs=NSB))
        p_w = ctx.enter_context(tc.tile_pool(name="w", bufs=3))
        p_qt = ctx.enter_context(tc.tile_pool(name="qt", bufs=2))
        p_v = ctx.enter_context(tc.tile_pool(name="v", bufs=KT))
        p_exp = ctx.enter_context(tc.tile_pool(name="exp", bufs=KT + 3))
        p_acc = ctx.enter_context(tc.tile_pool(name="acc", bufs=2))
        p_out = ctx.enter_context(tc.tile_pool(name="o", bufs=2))
        p_rc = ctx.enter_context(tc.tile_pool(name="rc", bufs=2))
        p_rbc = ctx.enter_context(tc.tile_pool(name="rbc", bufs=2))
        p_const = ctx.enter_context(tc.tile_pool(name="const", bufs=1))
        ps_mm = ctx.enter_context(tc.tile_pool(name="psmm", bufs=2, space="PSUM"))
        ps_pv = ctx.enter_context(tc.tile_pool(name="pspv", bufs=1, space="PSUM"))
        ps_pj = ctx.enter_context(tc.tile_pool(name="pspj", bufs=2, space="PSUM"))
        p_dram = ctx.enter_context(tc.tile_pool(name="dram", bufs=2, space="DRAM"))

        # ones [128,8]: the sum matmul writes the partition-sum of acc
        # replicated on 8 output partitions, so the PSUM->SBUF copy runs
        # 8 lanes wide instead of 1 (a [1,N] DVE op is ~1.2us).
        ones = p_const.tile([128, 8], F16, tag="ones")
        nc.vector.memset(ones[:], 1.0)
        ones1 = p_const.tile([1, 128], F16, tag="ones1")
        nc.vector.memset(ones1[:], 1.0)
        warm = p_const.tile([128, 512], F16, tag="warm")
        nc.vector.memset(warm[:], 0.0)

        # ---- input DMAs -------------------------------------------------
        # dst[p, d*512 + c] = src[d*128 + p, c]; one descriptor covers all
        # 8 d-tiles of a 512-column block. wv + x[:, :512] feed the first
        # V matmuls and are chunked per-d so accumulation starts as chunks
        # land; the rest ship as 512KB halves. Issues alternate between the
        # two HWDGE engines (SP, ACT) to halve the serialized issue time.
        xbig = [p_x.tile([128, ND * 512], F16, tag="x", name=f"xbig{i}") for i in range(NSB)]
        wbig = {n: p_w.tile([128, ND * 512], F16, tag=n, name=f"wbig_{n}") for n in ("wv", "wq", "wk")}
        w_aps = {"wv": wv_ap, "wq": wq_ap, "wk": wk_ap}

        def dma_w_chunk(eng, name, d0, d1, c0=0, c1=512):
            src = w_aps[name][d0 * 128:d1 * 128, c0:c1].rearrange(
                "(d p) c -> p d c", p=128)
            dst = wbig[name][:, d0 * 512:d1 * 512].rearrange(
                "p (d c) -> p d c", d=d1 - d0)[:, :, c0:c1]
            eng.dma_start(dst, src)

        def dma_x_chunk(eng, sb, d0, d1):
            src = xt_ap[d0 * 128:d1 * 128, sb * 512:(sb + 1) * 512].rearrange(
                "(d p) c -> p d c", p=128)
            eng.dma_start(xbig[sb][:, d0 * 512:d1 * 512], src)

        for d in range(ND):
            dma_w_chunk(nc.sync, "wv", d, d + 1, 0, 384)
            dma_x_chunk(nc.scalar, 0, d, d + 1)
        engs = (nc.sync, nc.scalar)
        for d in range(ND):
            dma_x_chunk(engs[d % 2], 1, d, d + 1)
        for q in range(4):
            dma_x_chunk(engs[q % 2], 2, 2 * q, 2 * q + 2)
        for q in range(4):
            dma_x_chunk(engs[q % 2], 3, 2 * q, 2 * q + 2)
        for q in range(4):
            dma_w_chunk(engs[q % 2], "wq", 2 * q, 2 * q + 2)
        for q in range(4):
            dma_w_chunk(engs[q % 2], "wk", 2 * q, 2 * q + 2)
        for q in range(2):
            dma_w_chunk(engs[q], "wv", 4 * q, 4 * q + 4, 384, 512)

        # PE p-state warmup: run the ramp on dummy matmuls while the first
        # input DMAs are in flight, so V starts at full clock.
        wps = ps_pj.tile([128, 512], F32, tag="proj", name="warmps")
        for _ in range(10):
            nc.tensor.matmul(wps[:], warm[:, :128], warm[:], start=True, stop=True)

        def xsl(d, sb):
            return xbig[sb][:, d * 512:(d + 1) * 512]

        def wsl(name, d, c0=0, c1=512):
            return wbig[name][:, d * 512 + c0:d * 512 + c1]

        # ---- V = x @ wv, [s, hd] layout, f16 ----------------------------
        # heads 0-2 here; head 3's V is deferred and dripped into the last
        # head's superblock, which is otherwise ACT(exp)-bound.
        vts = []
        for st in range(KT):
            ps = ps_mm.tile([128, 384], F32, tag="sT", name="vps")
            for d in range(ND):
                nc.tensor.matmul(
                    ps[:],
                    xsl(d, st // 4)[:, (st % 4) * 128:(st % 4 + 1) * 128],
                    wsl("wv", d, 0, 384),
                    start=(d == 0),
                    stop=(d == ND - 1),
                )
            vt = p_v.tile([128, 384], F16, tag="v")
            nc.vector.tensor_copy(vt[:], ps[:])
            vts.append(vt)
        vts3 = [None] * KT

        def vh3_step(st):
            ps = ps_pv.tile([128, 128], F32, tag="pv", name="vps3")
            for d in range(ND):
                nc.tensor.matmul(
                    ps[:],
                    xsl(d, st // 4)[:, (st % 4) * 128:(st % 4 + 1) * 128],
                    wsl("wv", d, 384, 512),
                    start=(d == 0),
                    stop=(d == ND - 1),
                )
            vt = p_v.tile([128, 128], F16, tag="v3")
            nc.vector.tensor_copy(vt[:], ps[:])
            vts3[st] = vt

        def proj_steps(h):
            """Yield once per PE-chunk of head h's q/k projections."""
            qt = p_qt.tile([128, S], F16, tag="qt")
            kt = p_qt.tile([128, S], F16, tag="kt")
            for dst, wname in ((qt, "wq"), (kt, "wk")):
                for sb in range(NSB):
                    ps = ps_pj.tile([128, 512], F32, tag="proj")
                    for d in range(ND):
                        nc.tensor.matmul(
                            ps[:],
                            wsl(wname, d, h * 128, (h + 1) * 128),
                            xsl(d, sb),
                            start=(d == 0),
                            stop=(d == ND - 1),
                        )
                        if d % 2 == 1:
                            yield None
                    nc.vector.tensor_copy(dst[:, sb * 512:(sb + 1) * 512], ps[:])
            while True:
                yield (qt, kt)

        def rc_chain(sms, deng, ceng=None):
            """sms: two [8,512] PSUM tiles holding the replicated key-sums.
            Returns the broadcast f16 reciprocal [128,1024]; DMAs issue on
            `deng` and the PSUM->SBUF copies on `ceng` so the two tail
            chains can run on separate engines."""
            sm_sb = p_rc.tile([8, 1024], F32, tag="sm_sb")
            if ceng is None:
                nc.vector.tensor_copy(sm_sb[:, :512], sms[0][:])
                nc.vector.tensor_copy(sm_sb[:, 512:], sms[1][:])
            else:
                ceng.copy(sm_sb[:, :512], sms[0][:])
                ceng.copy(sm_sb[:, 512:], sms[1][:])
            rc_in = p_rc.tile([128, 8], F32, tag="rc_in")
            deng.dma_start(rc_in[:], sm_sb[0:1, :], single_packet=True)
            rc_out = p_rc.tile([128, 8], F16, tag="rc_out")
            with nc.allow_low_precision(reason="f16 softmax denominator is ample for the 2e-2 gate"):
                nc.vector.reciprocal(rc_out[:], rc_in[:])
            r2dram = p_dram.tile([1, 1024], F16, tag="r2dram")
            deng.dma_start(
                r2dram[:].rearrange("a (p c) -> (a p) c", p=128), rc_out[:],
                single_packet=True,
            )
            rbc = p_rbc.tile([128, 1024], F16, tag="rbc")
            deng.dma_start(rbc[:], r2dram[0:1, :].to_broadcast((128, 1024)))
            return rbc

        def fold(acc, ets, k):
            # eager fold: acc = et0 + et1, then acc += etk
            if k == 1:
                nc.vector.tensor_add(acc[:], ets[0][:], ets[1][:])
            elif k > 1:
                nc.vector.tensor_add(acc[:], acc[:], ets[k][:])

        def qk_mm(st_ps, qt, kt, q0, k):
            for hf in range(2):
                nc.tensor.matmul(
                    st_ps[:, hf * 512:(hf + 1) * 512],
                    kt[:, k * 128:(k + 1) * 128],
                    qt[:, q0 + hf * 512:q0 + (hf + 1) * 512],
                    start=True,
                    stop=True,
                )

        def attention_block(h, qb, qt, kt, next_proj, tail_hook=None):
            pv = ps_pv.tile([128, 1024], F32, tag="pv")
            q0 = qb * 1024
            ets = {}
            acc = p_acc.tile([128, 1024], F16, tag="acc")

            def qk_step(k):
                st_ps = ps_mm.tile([128, 1024], F32, tag="sT")
                qk_mm(st_ps, qt, kt, q0, k)
                et = p_exp.tile([128, 1024], F16, tag="exp")
                nc.scalar.activation(
                    et[:], st_ps[:], mybir.ActivationFunctionType.Exp,
                    scale=SCALE,
                )
                ets[k] = et
                fold(acc, ets, k)

            def pv_step(k):
                et = ets.pop(k)
                for hf in range(2):
                    sl = slice(hf * 512, (hf + 1) * 512)
                    nc.tensor.matmul(
                        pv[:, sl],
                        vts[k][:, h * 128:(h + 1) * 128],
                        et[:, sl],
                        start=(k == 0),
                        stop=(k == KT - 1),
                    )

            def sum_chain():
                sms = []
                for hf in range(2):
                    sm = ps_pj.tile([8, 512], F32, tag="proj")
                    nc.tensor.matmul(
                        sm[:], ones[:], acc[:, hf * 512:(hf + 1) * 512],
                        start=True, stop=True,
                    )
                    sms.append(sm)
                return rc_chain(sms, nc.sync)

            LAG = 4
            for k in range(KT):
                qk_step(k)
                if k >= LAG:
                    pv_step(k - LAG)
                if next_proj is not None:
                    next(next_proj)
            if tail_hook is not None:
                tail_hook()
            for k in range(KT - LAG, KT - 2):
                pv_step(k)
            rbc = sum_chain()
            pv_step(KT - 2)
            pv_step(KT - 1)

            # free the pv PSUM bank immediately: the multiply waits on the
            # (DMA-latency) reciprocal chain, the copy does not.
            pv_sb = p_out.tile([128, 1024], F32, tag="pvsb")
            nc.vector.tensor_copy(pv_sb[:], pv[:])
            ob = p_out.tile([128, 1024], F16, tag="o")
            nc.gpsimd.tensor_mul(ob[:], pv_sb[:], rbc[:])
            nc.sync.dma_start(
                out_ap[h * 128:(h + 1) * 128, qb * 1024:(qb + 1) * 1024], ob[:]
            )

        def sb_make(qt, kt):
            """Shared state for the last head's superblock; created early so
            its first qk pair can be emitted inside the previous block's
            tail (keeps the ACT engine streaming across the boundary)."""
            ets0, ets1 = {}, {}
            acc0 = p_acc.tile([128, 1024], F16, tag="acc", name="acc0")
            acc1 = p_acc.tile([128, 1024], F16, tag="acc", name="acc1")

            def qk_step(qb, k):
                ets, acc = (ets0, acc0) if qb == 0 else (ets1, acc1)
                st_ps = ps_mm.tile([128, 1024], F32, tag="sT", name="sbst")
                qk_mm(st_ps, qt, kt, qb * 1024, k)
                et = p_exp.tile([128, 1024], F16, tag="exp", name="sbet")
                nc.scalar.activation(
                    et[:], st_ps[:], mybir.ActivationFunctionType.Exp,
                    scale=SCALE,
                )
                ets[k] = et
                fold(acc, ets, k)
            return {"ets0": ets0, "ets1": ets1, "acc0": acc0, "acc1": acc1,
                    "qk_step": qk_step}

        def attention_superblock(h, ctx):
            """Both query blocks of the last head, interleaved; qb1's PV runs
            as a trailing burst that covers both softmax-sum chains, so only
            the final multiply + store trail the kernel. qb0's PV accumulates
            in the (idle) projection PSUM pool as two half-tiles."""
            pv0 = [ps_pj.tile([128, 512], F32, tag="proj", name=f"pv0h{hf}")
                   for hf in range(2)]
            pv1 = ps_pv.tile([128, 1024], F32, tag="pv")
            ets0, ets1 = ctx["ets0"], ctx["ets1"]
            acc0, acc1 = ctx["acc0"], ctx["acc1"]
            qk_step = ctx["qk_step"]

            def pv0_step(k):
                et = ets0.pop(k)
                for hf in range(2):
                    sl = slice(hf * 512, (hf + 1) * 512)
                    nc.tensor.matmul(
                        pv0[hf][:],
                        vts3[k][:],
                        et[:, sl],
                        start=(k == 0),
                        stop=(k == KT - 1),
                    )

            def pv1_step(k):
                et = ets1[k]
                for hf in range(2):
                    sl = slice(hf * 512, (hf + 1) * 512)
                    nc.tensor.matmul(
                        pv1[:, sl],
                        vts3[k][:],
                        et[:, sl],
                        start=(k == 0),
                        stop=(k == KT - 1),
                    )

            def rc_flat(acc, deng, ceng):
                """key-sums of acc -> f16 reciprocal laid [1,1024] in SBUF."""
                sm = ps_mm.tile([8, 1024], F32, tag="sT", name="sm")
                for hf in range(2):
                    nc.tensor.matmul(
                        sm[:, hf * 512:(hf + 1) * 512], ones[:],
                        acc[:, hf * 512:(hf + 1) * 512],
                        start=True, stop=True,
                    )
                sm_sb = p_rc.tile([8, 1024], F32, tag="sm_sb")
                if ceng is None:
                    nc.vector.tensor_copy(sm_sb[:, :512], sm[:, :512])
                    nc.vector.tensor_copy(sm_sb[:, 512:], sm[:, 512:])
                else:
                    # split across ACT+DVE: this copy pair gates the last
                    # reciprocal chain of the kernel
                    ceng.copy(sm_sb[:, :512], sm[:, :512])
                    nc.vector.tensor_copy(sm_sb[:, 512:], sm[:, 512:])
                rc_in = p_rc.tile([128, 8], F32, tag="rc_in")
                deng.dma_start(rc_in[:], sm_sb[0:1, :], single_packet=True)
                rc_out = p_rc.tile([128, 8], F16, tag="rc_out")
                with nc.allow_low_precision(reason="f16 softmax denominator is ample for the 2e-2 gate"):
                    nc.vector.reciprocal(rc_out[:], rc_in[:])
                rcf = p_rc.tile([1, 1024], F16, tag="rcf")
                deng.dma_start(rcf[:], rc_out[:], single_packet=True)
                return rcf

            def rbc_mm(rcf):
                """broadcast the [1,1024] reciprocal to [128,1024] via PE."""
                rbc = ps_mm.tile([128, 1024], F32, tag="sT", name="rbc")
                for hf in range(2):
                    nc.tensor.matmul(
                        rbc[:, hf * 512:(hf + 1) * 512], ones1[:],
                        rcf[0:1, hf * 512:(hf + 1) * 512],
                        start=True, stop=True,
                    )
                return rbc

            vh3_step(0)
            qk_step(0, 1)
            vh3_step(1)
            qk_step(1, 1)
            for k in range(2, KT - 1):
                pv0_step(k - 2)
                qk_step(0, k)
                vh3_step(k)
                qk_step(1, k)
            qk_step(0, KT - 1)
            qk_step(1, KT - 1)
            vh3_step(KT - 1)
            pv0_step(KT - 3)
            pv0_step(KT - 2)
            pv0_step(KT - 1)
            pv1_step(0)
            pv1_step(1)
            rcf0 = rc_flat(acc0, nc.sync, None)
            pv1_step(2)
            pv1_step(3)
            # pv0 -> SBUF as soon as it stops (mid pv1-burst): the final
            # multiplies need SBUF operands (one-PSUM-operand rule), and
            # these copies must not queue on DVE behind the kernel tail.
            pv0_sb = p_out.tile([128, 1024], F32, tag="pvsb", name="pv0sb")
            for hf in range(2):
                nc.vector.tensor_copy(pv0_sb[:, hf * 512:(hf + 1) * 512], pv0[hf][:])
            rcf1 = rc_flat(acc1, nc.scalar, nc.scalar)
            for k in range(4, KT):
                pv1_step(k)

            pv1_sb = p_out.tile([128, 1024], F32, tag="pvsb", name="pv1sb")
            nc.vector.tensor_copy(pv1_sb[:], pv1[:])

            rbc0 = rbc_mm(rcf0)
            rbc1 = rbc_mm(rcf1)
            ob0 = p_out.tile([128, 1024], F16, tag="o")
            nc.vector.tensor_mul(ob0[:, :512], pv0_sb[:, :512], rbc0[:, :512])
            nc.vector.tensor_mul(ob0[:, 512:], pv0_sb[:, 512:], rbc0[:, 512:])
            nc.sync.dma_start(out_ap[h * 128:(h + 1) * 128, 0:512], ob0[:, :512])
            nc.sync.dma_start(out_ap[h * 128:(h + 1) * 128, 512:1024], ob0[:, 512:])
            ob1 = p_out.tile([128, 1024], F16, tag="o")
            nc.vector.tensor_mul(ob1[:, :512], pv1_sb[:, :512], rbc1[:, :512])
            nc.vector.tensor_mul(ob1[:, 512:], pv1_sb[:, 512:], rbc1[:, 512:])
            nc.scalar.dma_start(
                out_ap[h * 128:(h + 1) * 128, 1024:1536], ob1[:, :512])
            nc.scalar.dma_start(
                out_ap[h * 128:(h + 1) * 128, 1536:2048], ob1[:, 512:])

        # head 0's projections run serially (nothing to hide them under);
        # heads 1..3 project inside the previous head's attention loop.
        gen = proj_steps(0)
        res = None
        while not isinstance(res, tuple):
            res = next(gen)
        qt, kt = res
        sb_state = {}

        def sb_warm(gen):
            res = None
            while not isinstance(res, tuple):
                res = next(gen)
            qt3, kt3 = res
            ctx = sb_make(qt3, kt3)
            ctx["qk_step"](0, 0)
            ctx["qk_step"](1, 0)
            sb_state["ctx"] = ctx
            sb_state["qtkt"] = (qt3, kt3)

        for h in range(HPC):
            nxt = proj_steps(h + 1) if h + 1 < HPC else None
            if nxt is None:
                attention_superblock(h, sb_state["ctx"])
            else:
                for qb in range(2):
                    hook = None
                    if h == HPC - 2 and qb == 1:
                        hook = lambda: sb_warm(nxt)
                    attention_block(h, qb, qt, kt, nxt, tail_hook=hook)
            if nxt is not None:
                if h == HPC - 2:
                    qt, kt = sb_state["qtkt"]
                else:
                    res = None
                    while not isinstance(res, tuple):
                        res = next(nxt)
                    qt, kt = res


def _build():
    nc = bacc.Bacc(
        "TRN2",
        target_bir_lowering=False,
        debug=False,
        enable_asserts=False,
        num_devices=N_CORES,
    )
    xt_ap = nc.dram_tensor("xt", [D, S], F16, kind="ExternalInput").ap()
    wq_ap = nc.dram_tensor("wq", [D, DHG], F16, kind="ExternalInput").ap()
    wk_ap = nc.dram_tensor("wk", [D, DHG], F16, kind="ExternalInput").ap()
    wv_ap = nc.dram_tensor("wv", [D, DHG], F16, kind="ExternalInput").ap()
    out_ap = nc.dram_tensor("out", [DHG, S], F16, kind="ExternalOutput").ap()
    with tile.TileContext(nc) as tc:
        _emit(tc, nc, xt_ap, wq_ap, wk_ap, wv_ap, out_ap)
    nc.compile()
    return nc


def _shard_inputs(inputs):
    x = np.ascontiguousarray(np.asarray(inputs["input_embeddings"], dtype=np.float32))
    wq = np.asarray(inputs["w_query"], dtype=np.float32)
    wk = np.asarray(inputs["w_key"], dtype=np.float32)
    wv = np.asarray(inputs["w_value"], dtype=np.float32)

    def gather(w, g):
        # head h occupies the strided cols d = hd*8 + h; regroup head-major
        w4 = w.reshape(D, DH, H)[:, :, g * HPC:(g + 1) * HPC]   # (D, hd, hl)
        return np.ascontiguousarray(w4.transpose(0, 2, 1).reshape(D, DHG).astype(np.float16))

    in_maps = []
    for c in range(N_CORES):
        b, g = divmod(c, 2)
        in_maps.append(
            {
                "xt": np.ascontiguousarray(x[b].T.astype(np.float16)),
                "wq": gather(wq, g),
                "wk": gather(wk, g),
                "wv": gather(wv, g),
            }
        )
    return in_maps


def kernel(**inputs):
    nc = _CACHE.get("nc")
    if nc is None:
        nc = _CACHE["nc"] = _build()
    in_maps = _shard_inputs(inputs)
    res = run_bass_kernel_spmd(
        nc, in_maps, core_ids=list(range(N_CORES)), trace=TRACE
    )
    _CACHE["last_result"] = res
    out = np.empty((B, S, DH, H), dtype=np.float32)
    for c in range(N_CORES):
        b, g = divmod(c, 2)
        o = res.results[c]["out"].astype(np.float32).reshape(HPC, DH, S)  # (hl, hd, s)
        out[b, :, :, g * HPC:(g + 1) * HPC] = o.transpose(2, 1, 0)
    return out.reshape(B, S, D)


# revision 48
# speedup vs baseline: 1.0034x; 1.0034x over previous
"""Multi-head self-attention (B=4, S=2048, D=1024, H=8) on 8 TRN2 NeuronCores.

Sharding: core c -> batch b=c//2, head-group g=c%2 (4 heads/core).
Each core computes its 4 heads' attention output [512, 2048] (transposed,
head-major, f16); the host gathers/reassembles the full [B, S, D] output.

Notes on the math: the reference adds the source mask per-QUERY (constant
along the key axis) before a softmax over keys, so the mask cancels exactly;
encoder_output_embedding and the target mask are unused by the reference.
The kernel therefore computes pure softmax(q k^T / sqrt(dh)) v.

Schedule (what got it from ~246us to ~228us):
- Input DMAs batched into few descriptors split across the two HWDGE
  engines (SP + ACT): dma_start issue is ~650ns serialized per engine;
  56 small DMAs made the PE idle ~11us at the start. A few dummy
  matmuls warm the PE p-state while the first tiles land.
- The 1/sqrt(dh) scale is folded into the exp activation's scale.
- Softmax denominators: eager DVE folds of the exp tiles; ones-matmul
  partition-reduce (8 replicated rows so the PSUM->SBUF copy runs 8
  lanes wide); [128,8] repartition DMA; DVE reciprocal (f16);
  departition DMA; broadcast back as [128,1024] (DRAM-bounce DMA for
  pipelined blocks, PE ones-matmul broadcast for the final chains).
- PV PSUM is copied to SBUF as soon as it stops so the bank frees
  without waiting on the reciprocal chain; the final multiply runs on
  GpSimd (both operands SBUF) to avoid DVE head-of-line blocking.
- Head 3's V projection is deferred out of the V phase and dripped into
  the last head's "superblock" (its two query blocks interleaved),
  which is otherwise exp(ACT)-bound; the superblock's first qk pair is
  emitted inside the previous block's tail to keep ACT streaming.
- Heads 1..3 project inside the previous head's attention loop
  (2 matmuls dripped per key step).
- Output is written f16 (half the DMA bytes; rel err stays ~6e-4).
"""
import math
from contextlib import ExitStack

import numpy as np

import concourse.bacc as bacc
import concourse.tile as tile
from concourse import mybir
from concourse.bass_utils import run_bass_kernel_spmd

N_CORES = 8
B, S, D, H = 4, 2048, 1024, 8
DH = 128                    # head dim
HPC = 4                     # heads per core
DHG = HPC * DH              # 512: projected width per core
SCALE = 1.0 / math.sqrt(DH)

F32 = mybir.dt.float32
F16 = mybir.dt.float16

TRACE = False               # test.py flips this for profiling runs
_CACHE = {}

KT = S // 128               # 16 key tiles
ND = D // 128               # 8 contraction tiles
NSB = S // 512              # 4 column blocks of x


def _emit(tc, nc, xt_ap, wq_ap, wk_ap, wv_ap, out_ap):
    with ExitStack() as ctx:
        p_x = ctx.enter_context(tc.tile_pool(name="x", bufs=NSB))
        p_w = ctx.enter_context(tc.tile_pool(name="w", bufs=3))
        p_qt = ctx.enter_context(tc.tile_pool(name="qt", bufs=2))
        p_v = ctx.enter_context(tc.tile_pool(name="v", bufs=KT))
        p_exp = ctx.enter_context(tc.tile_pool(name="exp", bufs=KT + 3))
        p_acc = ctx.enter_context(tc.tile_pool(name="acc", bufs=2))
        p_out = ctx.enter_context(tc.tile_pool(name="o", bufs=2))
        p_rc = ctx.enter_context(tc.tile_pool(name="rc", bufs=2))
        p_rbc = ctx.enter_context(tc.tile_pool(name="rbc", bufs=2))
        p_const = ctx.enter_context(tc.tile_pool(name="const", bufs=1))
        ps_mm = ctx.enter_context(tc.tile_pool(name="psmm", bufs=2, space="PSUM"))
        ps_pv = ctx.enter_context(tc.tile_pool(name="pspv", bufs=1, space="PSUM"))
        ps_pj = ctx.enter_context(tc.tile_pool(name="pspj", bufs=2, space="PSUM"))
        p_dram = ctx.enter_context(tc.tile_pool(name="dram", bufs=2, space="DRAM"))

        # ones [128,8]: the sum matmul writes the partition-sum of acc
        # replicated on 8 output partitions, so the PSUM->SBUF copy runs
        # 8 lanes wide instead of 1 (a [1,N] DVE op is ~1.2us).
        ones = p_const.tile([128, 8], F16, tag="ones")
        nc.vector.memset(ones[:], 1.0)
        ones1 = p_const.tile([1, 128], F16, tag="ones1")
        nc.vector.memset(ones1[:], 1.0)
        warm = p_const.tile([128, 512], F16, tag="warm")
        nc.vector.memset(warm[:], 0.0)

        # ---- input DMAs -------------------------------------------------
        # dst[p, d*512 + c] = src[d*128 + p, c]; one descriptor covers all
        # 8 d-tiles of a 512-column block. wv + x[:, :512] feed the first
        # V matmuls and are chunked per-d so accumulation starts as chunks
        # land; the rest ship as 512KB halves. Issues alternate between the
        # two HWDGE engines (SP, ACT) to halve the serialized issue time.
        xbig = [p_x.tile([128, ND * 512], F16, tag="x", name=f"xbig{i}") for i in range(NSB)]
        wbig = {n: p_w.tile([128, ND * 512], F16, tag=n, name=f"wbig_{n}") for n in ("wv", "wq", "wk")}
        w_aps = {"wv": wv_ap, "wq": wq_ap, "wk": wk_ap}

        def dma_w_chunk(eng, name, d0, d1, c0=0, c1=512):
            src = w_aps[name][d0 * 128:d1 * 128, c0:c1].rearrange(
                "(d p) c -> p d c", p=128)
            dst = wbig[name][:, d0 * 512:d1 * 512].rearrange(
                "p (d c) -> p d c", d=d1 - d0)[:, :, c0:c1]
            eng.dma_start(dst, src)

        def dma_x_chunk(eng, sb, d0, d1):
            src = xt_ap[d0 * 128:d1 * 128, sb * 512:(sb + 1) * 512].rearrange(
                "(d p) c -> p d c", p=128)
            eng.dma_start(xbig[sb][:, d0 * 512:d1 * 512], src)

        for d in range(ND):
            dma_w_chunk(nc.sync, "wv", d, d + 1, 0, 384)
            dma_x_chunk(nc.scalar, 0, d, d + 1)
        engs = (nc.sync, nc.scalar)
        for d in range(ND):
            dma_x_chunk(engs[d % 2], 1, d, d + 1)
        for q in range(4):
            dma_x_chunk(engs[q % 2], 2, 2 * q, 2 * q + 2)
        for q in range(4):
            dma_x_chunk(engs[q % 2], 3, 2 * q, 2 * q + 2)
        for q in range(4):
            dma_w_chunk(engs[q % 2], "wq", 2 * q, 2 * q + 2)
        for q in range(4):
            dma_w_chunk(engs[q % 2], "wk", 2 * q, 2 * q + 2)
        for q in range(2):
            dma_w_chunk(engs[q], "wv", 4 * q, 4 * q + 4, 384, 512)

        # PE p-state warmup: run the ramp on dummy matmuls while the first
        # input DMAs are in flight, so V starts at full clock.
        wps = ps_pj.tile([128, 512], F32, tag="proj", name="warmps")
        for _ in range(10):
            nc.tensor.matmul(wps[:], warm[:, :128], warm[:], start=True, stop=True)

        def xsl(d, sb):
            return xbig[sb][:, d * 512:(d + 1) * 512]

        def wsl(name, d, c0=0, c1=512):
            return wbig[name][:, d * 512 + c0:d * 512 + c1]

        # ---- V = x @ wv, [s, hd] layout, f16 ----------------------------
        # heads 0-2 here; head 3's V is deferred and dripped into the last
        # head's superblock, which is otherwise ACT(exp)-bound.
        vts = []
        for st in range(KT):
            ps = ps_mm.tile([128, 384], F32, tag="sT", name="vps")
            for d in range(ND):
                nc.tensor.matmul(
                    ps[:],
                    xsl(d, st // 4)[:, (st % 4) * 128:(st % 4 + 1) * 128],
                    wsl("wv", d, 0, 384),
                    start=(d == 0),
                    stop=(d == ND - 1),
                )
            vt = p_v.tile([128, 384], F16, tag="v")
            nc.vector.tensor_copy(vt[:], ps[:])
            vts.append(vt)
        vts3 = [None] * KT

        def vh3_step(st):
            ps = ps_pv.tile([128, 128], F32, tag="pv", name="vps3")
            for d in range(ND):
                nc.tensor.matmul(
                    ps[:],
                    xsl(d, st // 4)[:, (st % 4) * 128:(st % 4 + 1) * 128],
                    wsl("wv", d, 384, 512),
                    start=(d == 0),
                    stop=(d == ND - 1),
                )
            vt = p_v.tile([128, 128], F16, tag="v3")
            nc.vector.tensor_copy(vt[:], ps[:])
            vts3[st] = vt

        def proj_steps(h):
            """Yield once per PE-chunk of head h's q/k projections."""
            qt = p_qt.tile([128, S], F16, tag="qt")
            kt = p_qt.tile([128, S], F16, tag="kt")
            for dst, wname in ((qt, "wq"), (kt, "wk")):
                for sb in range(NSB):
                    ps = ps_pj.tile([128, 512], F32, tag="proj")
                    for d in range(ND):
                        nc.tensor.matmul(
                            ps[:],
                            wsl(wname, d, h * 128, (h + 1) * 128),
                            xsl(d, sb),
                            start=(d == 0),
                            stop=(d == ND - 1),
                        )
                        if d % 2 == 1:
                            yield None
                    nc.vector.tensor_copy(dst[:, sb * 512:(sb + 1) * 512], ps[:])
            while True:
                yield (qt, kt)

        def rc_chain(sms, deng, ceng=None):
            """sms: two [8,512] PSUM tiles holding the replicated key-sums.
            Returns the broadcast f16 reciprocal [128,1024]; DMAs issue on
            `deng` and the PSUM->SBUF copies on `ceng` so the two tail
            chains can run on separate engines."""
            sm_sb = p_rc.tile([8, 1024], F32, tag="sm_sb")
            if ceng is None:
                nc.vector.tensor_copy(sm_sb[:, :512], sms[0][:])
                nc.vector.tensor_copy(sm_sb[:, 512:], sms[1][:])
            else:
                ceng.copy(sm_sb[:, :512], sms[0][:])
                ceng.copy(sm_sb[:, 512:], sms[1][:])
            rc_in = p_rc.tile([128, 8], F32, tag="rc_in")
            deng.dma_start(rc_in[:], sm_sb[0:1, :], single_packet=True)
            rc_out = p_rc.tile([128, 8], F16, tag="rc_out")
            with nc.allow_low_precision(reason="f16 softmax denominator is ample for the 2e-2 gate"):
                nc.vector.reciprocal(rc_out[:], rc_in[:])
            r2dram = p_dram.tile([1, 1024], F16, tag="r2dram")
            deng.dma_start(
                r2dram[:].rearrange("a (p c) -> (a p) c", p=128), rc_out[:],
                single_packet=True,
            )
            rbc = p_rbc.tile([128, 1024], F16, tag="rbc")
            deng.dma_start(rbc[:], r2dram[0:1, :].to_broadcast((128, 1024)))
            return rbc

        def fold(acc, ets, k):
            # eager fold: acc = et0 + et1, then acc += etk
            if k == 1:
                nc.vector.tensor_add(acc[:], ets[0][:], ets[1][:])
            elif k > 1:
                nc.vector.tensor_add(acc[:], acc[:], ets[k][:])

        def qk_mm(st_ps, qt, kt, q0, k):
            for hf in range(2):
                nc.tensor.matmul(
                    st_ps[:, hf * 512:(hf + 1) * 512],
                    kt[:, k * 128:(k + 1) * 128],
                    qt[:, q0 + hf * 512:q0 + (hf + 1) * 512],
                    start=True,
                    stop=True,
                )

        def attention_block(h, qb, qt, kt, next_proj, tail_hook=None):
            pv = ps_pv.tile([128, 1024], F32, tag="pv")
            q0 = qb * 1024
            ets = {}
            acc = p_acc.tile([128, 1024], F16, tag="acc")

            def qk_step(k):
                st_ps = ps_mm.tile([128, 1024], F32, tag="sT")
                qk_mm(st_ps, qt, kt, q0, k)
                et = p_exp.tile([128, 1024], F16, tag="exp")
                nc.scalar.activation(
                    et[:], st_ps[:], mybir.ActivationFunctionType.Exp,
                    scale=SCALE,
                )
                ets[k] = et
                fold(acc, ets, k)

            def pv_step(k):
                et = ets.pop(k)
                for hf in range(2):
                    sl = slice(hf * 512, (hf + 1) * 512)
                    nc.tensor.matmul(
                        pv[:, sl],
                        vts[k][:, h * 128:(h + 1) * 128],
                        et[:, sl],
                        start=(k == 0),
                        stop=(k == KT - 1),
                    )

            def sum_chain():
                sms = []
                for hf in range(2):
                    sm = ps_pj.tile([8, 512], F32, tag="proj")
                    nc.tensor.matmul(
                        sm[:], ones[:], acc[:, hf * 512:(hf + 1) * 512],
                        start=True, stop=True,
                    )
                    sms.append(sm)
                return rc_chain(sms, nc.sync)

            LAG = 4
            for k in range(KT):
                qk_step(k)
                if k >= LAG:
                    pv_step(k - LAG)
                if next_proj is not None:
                    next(next_proj)
            if tail_hook is not None:
                tail_hook()
            for k in range(KT - LAG, KT - 2):
                pv_step(k)
            rbc = sum_chain()
            pv_step(KT - 2)
            pv_step(KT - 1)

            # free the pv PSUM bank immediately: the multiply waits on the
            # (DMA-latency) reciprocal chain, the copy does not.
            pv_sb = p_out.tile([128, 1024], F32, tag="pvsb")
            nc.vector.tensor_copy(pv_sb[:], pv[:])
            ob = p_out.tile([128, 1024], F16, tag="o")
            nc.gpsimd.tensor_mul(ob[:], pv_sb[:], rbc[:])
            nc.sync.dma_start(
                out_ap[h * 128:(h + 1) * 128, qb * 1024:(qb + 1) * 1024], ob[:]
            )

        def sb_make(qt, kt):
            """Shared state for the last head's superblock; created early so
            its first qk pair can be emitted inside the previous block's
            tail (keeps the ACT engine streaming across the boundary)."""
            ets0, ets1 = {}, {}
            acc0 = p_acc.tile([128, 1024], F16, tag="acc", name="acc0")
            acc1 = p_acc.tile([128, 1024], F16, tag="acc", name="acc1")

            def qk_step(qb, k):
                ets, acc = (ets0, acc0) if qb == 0 else (ets1, acc1)
                st_ps = ps_mm.tile([128, 1024], F32, tag="sT", name="sbst")
                qk_mm(st_ps, qt, kt, qb * 1024, k)
                et = p_exp.tile([128, 1024], F16, tag="exp", name="sbet")
                nc.scalar.activation(
                    et[:], st_ps[:], mybir.ActivationFunctionType.Exp,
                    scale=SCALE,
                )
                ets[k] = et
                fold(acc, ets, k)
            return {"ets0": ets0, "ets1": ets1, "acc0": acc0, "acc1": acc1,
                    "qk_step": qk_step}

        def attention_superblock(h, ctx):
            """Both query blocks of the last head, interleaved; qb1's PV runs
            as a trailing burst that covers both softmax-sum chains, so only
            the final multiply + store trail the kernel. qb0's PV accumulates
            in the (idle) projection PSUM pool as two half-tiles."""
            pv0 = [ps_pj.tile([128, 512], F32, tag="proj", name=f"pv0h{hf}")
                   for hf in range(2)]
            pv1 = ps_pv.tile([128, 1024], F32, tag="pv")
            ets0, ets1 = ctx["ets0"], ctx["ets1"]
            acc0, acc1 = ctx["acc0"], ctx["acc1"]
            qk_step = ctx["qk_step"]

            def pv0_step(k):
                et = ets0.pop(k)
                for hf in range(2):
                    sl = slice(hf * 512, (hf + 1) * 512)
                    nc.tensor.matmul(
                        pv0[hf][:],
                        vts3[k][:],
                        et[:, sl],
                        start=(k == 0),
                        stop=(k == KT - 1),
                    )

            def pv1_step(k):
                et = ets1[k]
                for hf in range(2):
                    sl = slice(hf * 512, (hf + 1) * 512)
                    nc.tensor.matmul(
                        pv1[:, sl],
                        vts3[k][:],
                        et[:, sl],
                        start=(k == 0),
                        stop=(k == KT - 1),
                    )

            def rc_flat(acc, deng, ceng):
                """key-sums of acc -> f16 reciprocal laid [1,1024] in SBUF."""
                sm = ps_mm.tile([8, 1024], F32, tag="sT", name="sm")
                for hf in range(2):
                    nc.tensor.matmul(
                        sm[:, hf * 512:(hf + 1) * 512], ones[:],
                        acc[:, hf * 512:(hf + 1) * 512],
                        start=True, stop=True,
                    )
                sm_sb = p_rc.tile([8, 1024], F32, tag="sm_sb")
                if ceng is None:
                    nc.vector.tensor_copy(sm_sb[:, :512], sm[:, :512])
                    nc.vector.tensor_copy(sm_sb[:, 512:], sm[:, 512:])
                else:
                    # split across ACT+DVE: this copy pair gates the last
                    # reciprocal chain of the kernel
                    ceng.copy(sm_sb[:, :512], sm[:, :512])
                    nc.vector.tensor_copy(sm_sb[:, 512:], sm[:, 512:])
                rc_in = p_rc.tile([128, 8], F32, tag="rc_in")
                deng.dma_start(rc_in[:], sm_sb[0:1, :], single_packet=True)
                rc_out = p_rc.tile([128, 8], F16, tag="rc_out")
                with nc.allow_low_precision(reason="f16 softmax denominator is ample for the 2e-2 gate"):
                    nc.vector.reciprocal(rc_out[:], rc_in[:])
                rcf = p_rc.tile([1, 1024], F16, tag="rcf")
                deng.dma_start(rcf[:], rc_out[:], single_packet=True)
                return rcf

            def rbc_mm(rcf):
                """broadcast the [1,1024] reciprocal to [128,1024] via PE."""
                rbc = ps_mm.tile([128, 1024], F32, tag="sT", name="rbc")
                for hf in range(2):
                    nc.tensor.matmul(
                        rbc[:, hf * 512:(hf + 1) * 512], ones1[:],
                        rcf[0:1, hf * 512:(hf + 1) * 512],
                        start=True, stop=True,
                    )
                return rbc

            vh3_step(0)
            qk_step(0, 1)
            vh3_step(1)
            qk_step(1, 1)
            for k in range(2, KT - 1):
                pv0_step(k - 2)
                qk_step(0, k)
                vh3_step(k)
                qk_step(1, k)
            qk_step(0, KT - 1)
            qk_step(1, KT - 1)
            vh3_step(KT - 1)
            pv0_step(KT - 3)
            pv0_step(KT - 2)
            pv0_step(KT - 1)
            pv1_step(0)
            pv1_step(1)
            rcf0 = rc_flat(acc0, nc.sync, None)
            pv1_step(2)
            pv1_step(3)
            # pv0 -> SBUF as soon as it stops (mid pv1-burst): the final
            # multiplies need SBUF operands (one-PSUM-operand rule), and
            # these copies must not queue on DVE behind the kernel tail.
            pv0_sb = p_out.tile([128, 1024], F32, tag="pvsb", name="pv0sb")
            for hf in range(2):
                nc.vector.tensor_copy(pv0_sb[:, hf * 512:(hf + 1) * 512], pv0[hf][:])
            rcf1 = rc_flat(acc1, nc.scalar, nc.scalar)
            for k in range(4, KT):
                pv1_step(k)

            pv1_sb = p_out.tile([128, 1024], F32, tag="pvsb", name="pv1sb")
            nc.vector.tensor_copy(pv1_sb[:], pv1[:])

            rbc0 = rbc_mm(rcf0)
            rbc1 = rbc_mm(rcf1)
            ob0 = p_out.tile([128, 1024], F16, tag="o")
            nc.vector.tensor_mul(ob0[:, :512], pv0_sb[:, :512], rbc0[:, :512])
            nc.vector.tensor_mul(ob0[:, 512:], pv0_sb[:, 512:], rbc0[:, 512:])
            nc.sync.dma_start(out_ap[h * 128:(h + 1) * 128, 0:512], ob0[:, :512])
            nc.sync.dma_start(out_ap[h * 128:(h + 1) * 128, 512:1024], ob0[:, 512:])
            ob1 = p_out.tile([128, 1024], F16, tag="o")
            nc.vector.tensor_mul(ob1[:, :512], pv1_sb[:, :512], rbc1[:, :512])
            nc.vector.tensor_mul(ob1[:, 512:], pv1_sb[:, 512:], rbc1[:, 512:])
            nc.scalar.dma_start(
                out_ap[h * 128:(h + 1) * 128, 1024:1536], ob1[:, :512])
            nc.scalar.dma_start(
                out_ap[h * 128:(h + 1) * 128, 1536:2048], ob1[:, 512:])

        # head 0's projections run serially (nothing to hide them under);
        # heads 1..3 project inside the previous head's attention loop.
        gen = proj_steps(0)
        res = None
        while not isinstance(res, tuple):
            res = next(gen)
        qt, kt = res
        sb_state = {}

        def sb_warm(gen):
            res = None
            while not isinstance(res, tuple):
                res = next(gen)
            qt3, kt3 = res
            ctx = sb_make(qt3, kt3)
            ctx["qk_step"](0, 0)
            ctx["qk_step"](1, 0)
            sb_state["ctx"] = ctx
            sb_state["qtkt"] = (qt3, kt3)

        for h in range(HPC):
            nxt = proj_steps(h + 1) if h + 1 < HPC else None
            if nxt is None:
                attention_superblock(h, sb_state["ctx"])
            else:
                for qb in range(2):
                    hook = None
                    if h == HPC - 2 and qb == 1:
                        hook = lambda: sb_warm(nxt)
                    attention_block(h, qb, qt, kt, nxt, tail_hook=hook)
            if nxt is not None:
                if h == HPC - 2:
                    qt, kt = sb_state["qtkt"]
                else:
                    res = None
                    while not isinstance(res, tuple):
                        res = next(nxt)
                    qt, kt = res


def _build():
    nc = bacc.Bacc(
        "TRN2",
        target_bir_lowering=False,
        debug=False,
        enable_asserts=False,
        num_devices=N_CORES,
    )
    xt_ap = nc.dram_tensor("xt", [D, S], F16, kind="ExternalInput").ap()
    wq_ap = nc.dram_tensor("wq", [D, DHG], F16, kind="ExternalInput").ap()
    wk_ap = nc.dram_tensor("wk", [D, DHG], F16, kind="ExternalInput").ap()
    wv_ap = nc.dram_tensor("wv", [D, DHG], F16, kind="ExternalInput").ap()
    out_ap = nc.dram_tensor("out", [DHG, S], F16, kind="ExternalOutput").ap()
    with tile.TileContext(nc) as tc:
        _emit(tc, nc, xt_ap, wq_ap, wk_ap, wv_ap, out_ap)
    nc.compile()
    return nc


def _shard_inputs(inputs):
    x = np.ascontiguousarray(np.asarray(inputs["input_embeddings"], dtype=np.float32))
    wq = np.asarray(inputs["w_query"], dtype=np.float32)
    wk = np.asarray(inputs["w_key"], dtype=np.float32)
    wv = np.asarray(inputs["w_value"], dtype=np.float32)

    def gather(w, g):
        # head h occupies the strided cols d = hd*8 + h; regroup head-major
        w4 = w.reshape(D, DH, H)[:, :, g * HPC:(g + 1) * HPC]   # (D, hd, hl)
        return np.ascontiguousarray(w4.transpose(0, 2, 1).reshape(D, DHG).astype(np.float16))

    in_maps = []
    for c in range(N_CORES):
        b, g = divmod(c, 2)
        in_maps.append(
            {
                "xt": np.ascontiguousarray(x[b].T.astype(np.float16)),
                "wq": gather(wq, g),
                "wk": gather(wk, g),
                "wv": gather(wv, g),
            }
        )
    return in_maps


def kernel(**inputs):
    nc = _CACHE.get("nc")
    if nc is None:
        nc = _CACHE["nc"] = _build()
    in_maps = _shard_inputs(inputs)
    res = run_bass_kernel_spmd(
        nc, in_maps, core_ids=list(range(N_CORES)), trace=TRACE
    )
    _CACHE["last_result"] = res
    out = np.empty((B, S, DH, H), dtype=np.float32)
    for c in range(N_CORES):
        b, g = divmod(c, 2)
        o = res.results[c]["out"].astype(np.float32).reshape(HPC, DH, S)  # (hl, hd, s)
        out[b, :, :, g * HPC:(g + 1) * HPC] = o.transpose(2, 1, 0)
    return out.reshape(B, S, D)


# revision 51
# speedup vs baseline: 1.0078x; 1.0044x over previous
"""Multi-head self-attention (B=4, S=2048, D=1024, H=8) on 8 TRN2 NeuronCores.

Sharding: core c -> batch b=c//2, head-group g=c%2 (4 heads/core).
Each core computes its 4 heads' attention output [512, 2048] (transposed,
head-major, f16); the host gathers/reassembles the full [B, S, D] output.

Notes on the math: the reference adds the source mask per-QUERY (constant
along the key axis) before a softmax over keys, so the mask cancels exactly;
encoder_output_embedding and the target mask are unused by the reference.
The kernel therefore computes pure softmax(q k^T / sqrt(dh)) v.

Schedule (what got it from ~246us to ~228us):
- Input DMAs batched into few descriptors split across the two HWDGE
  engines (SP + ACT): dma_start issue is ~650ns serialized per engine;
  56 small DMAs made the PE idle ~11us at the start. A few dummy
  matmuls warm the PE p-state while the first tiles land.
- The 1/sqrt(dh) scale is folded into the exp activation's scale.
- Softmax denominators: eager DVE folds of the exp tiles; ones-matmul
  partition-reduce (8 replicated rows so the PSUM->SBUF copy runs 8
  lanes wide); [128,8] repartition DMA; DVE reciprocal (f16);
  departition DMA; broadcast back as [128,1024] (DRAM-bounce DMA for
  pipelined blocks, PE ones-matmul broadcast for the final chains).
- PV PSUM is copied to SBUF as soon as it stops so the bank frees
  without waiting on the reciprocal chain; the final multiply runs on
  GpSimd (both operands SBUF) to avoid DVE head-of-line blocking.
- Head 3's V projection is deferred out of the V phase and dripped into
  the last head's "superblock" (its two query blocks interleaved),
  which is otherwise exp(ACT)-bound; the superblock's first qk pair is
  emitted inside the previous block's tail to keep ACT streaming.
- Heads 1..3 project inside the previous head's attention loop
  (2 matmuls dripped per key step).
- Output is written f16 (half the DMA bytes; rel err stays ~6e-4).
"""
import math
from contextlib import ExitStack

import numpy as np

import concourse.bacc as bacc
import concourse.tile as tile
from concourse import mybir
from concourse.bass_utils import run_bass_kernel_spmd

N_CORES = 8
B, S, D, H = 4, 2048, 1024, 8
DH = 128                    # head dim
HPC = 4                     # heads per core
DHG = HPC * DH              # 512: projected width per core
SCALE = 1.0 / math.sqrt(DH)

F32 = mybir.dt.float32
F16 = mybir.dt.float16

TRACE = False               # test.py flips this for profiling runs
_CACHE = {}

KT = S // 128               # 16 key tiles
ND = D // 128               # 8 contraction tiles
NSB = S // 512              # 4 column blocks of x


def _emit(tc, nc, xt_ap, wq_ap, wk_ap, wv_ap, out_ap):
    with ExitStack() as ctx:
        p_x = ctx.enter_context(tc.tile_pool(name="x", bufs=NSB))
        p_w = ctx.enter_context(tc.tile_pool(name="w", bufs=3))
        p_qt = ctx.enter_context(tc.tile_pool(name="qt", bufs=2))
        p_v = ctx.enter_context(tc.tile_pool(name="v", bufs=KT))
        p_exp = ctx.enter_context(tc.tile_pool(name="exp", bufs=KT + 5))
        p_acc = ctx.enter_context(tc.tile_pool(name="acc", bufs=2))
        p_out = ctx.enter_context(tc.tile_pool(name="o", bufs=2))
        p_rc = ctx.enter_context(tc.tile_pool(name="rc", bufs=2))
        p_rbc = ctx.enter_context(tc.tile_pool(name="rbc", bufs=2))
        p_const = ctx.enter_context(tc.tile_pool(name="const", bufs=1))
        ps_mm = ctx.enter_context(tc.tile_pool(name="psmm", bufs=2, space="PSUM"))
        ps_pv = ctx.enter_context(tc.tile_pool(name="pspv", bufs=1, space="PSUM"))
        ps_pj = ctx.enter_context(tc.tile_pool(name="pspj", bufs=2, space="PSUM"))
        p_dram = ctx.enter_context(tc.tile_pool(name="dram", bufs=2, space="DRAM"))

        # ones [128,8]: the sum matmul writes the partition-sum of acc
        # replicated on 8 output partitions, so the PSUM->SBUF copy runs
        # 8 lanes wide instead of 1 (a [1,N] DVE op is ~1.2us).
        ones = p_const.tile([128, 8], F16, tag="ones")
        nc.vector.memset(ones[:], 1.0)
        ones1 = p_const.tile([1, 128], F16, tag="ones1")
        nc.vector.memset(ones1[:], 1.0)
        warm = p_const.tile([128, 512], F16, tag="warm")
        nc.vector.memset(warm[:], 0.0)

        # ---- input DMAs -------------------------------------------------
        # dst[p, d*512 + c] = src[d*128 + p, c]; one descriptor covers all
        # 8 d-tiles of a 512-column block. wv + x[:, :512] feed the first
        # V matmuls and are chunked per-d so accumulation starts as chunks
        # land; the rest ship as 512KB halves. Issues alternate between the
        # two HWDGE engines (SP, ACT) to halve the serialized issue time.
        xbig = [p_x.tile([128, ND * 512], F16, tag="x", name=f"xbig{i}") for i in range(NSB)]
        wbig = {n: p_w.tile([128, ND * 512], F16, tag=n, name=f"wbig_{n}") for n in ("wv", "wq", "wk")}
        w_aps = {"wv": wv_ap, "wq": wq_ap, "wk": wk_ap}

        def dma_w_chunk(eng, name, d0, d1, c0=0, c1=512):
            src = w_aps[name][d0 * 128:d1 * 128, c0:c1].rearrange(
                "(d p) c -> p d c", p=128)
            dst = wbig[name][:, d0 * 512:d1 * 512].rearrange(
                "p (d c) -> p d c", d=d1 - d0)[:, :, c0:c1]
            eng.dma_start(dst, src)

        def dma_x_chunk(eng, sb, d0, d1):
            src = xt_ap[d0 * 128:d1 * 128, sb * 512:(sb + 1) * 512].rearrange(
                "(d p) c -> p d c", p=128)
            eng.dma_start(xbig[sb][:, d0 * 512:d1 * 512], src)

        for d in range(ND):
            dma_w_chunk(nc.sync, "wv", d, d + 1, 0, 384)
            dma_x_chunk(nc.scalar, 0, d, d + 1)
        engs = (nc.sync, nc.scalar)
        for d in range(ND):
            dma_x_chunk(engs[d % 2], 1, d, d + 1)
        for q in range(4):
            dma_x_chunk(engs[q % 2], 2, 2 * q, 2 * q + 2)
        for q in range(4):
            dma_x_chunk(engs[q % 2], 3, 2 * q, 2 * q + 2)
        for q in range(4):
            dma_w_chunk(engs[q % 2], "wq", 2 * q, 2 * q + 2)
        for q in range(4):
            dma_w_chunk(engs[q % 2], "wk", 2 * q, 2 * q + 2)
        for q in range(2):
            dma_w_chunk(engs[q], "wv", 4 * q, 4 * q + 4, 384, 512)

        # PE p-state warmup: run the ramp on dummy matmuls while the first
        # input DMAs are in flight, so V starts at full clock.
        wps = ps_pj.tile([128, 512], F32, tag="proj", name="warmps")
        for _ in range(10):
            nc.tensor.matmul(wps[:], warm[:, :128], warm[:], start=True, stop=True)

        def xsl(d, sb):
            return xbig[sb][:, d * 512:(d + 1) * 512]

        def wsl(name, d, c0=0, c1=512):
            return wbig[name][:, d * 512 + c0:d * 512 + c1]

        # ---- V = x @ wv, [s, hd] layout, f16 ----------------------------
        # heads 0-2 here; head 3's V is deferred and dripped into the last
        # head's superblock, which is otherwise ACT(exp)-bound.
        vts = []
        for st in range(KT):
            ps = ps_mm.tile([128, 384], F32, tag="sT", name="vps")
            for d in range(ND):
                nc.tensor.matmul(
                    ps[:],
                    xsl(d, st // 4)[:, (st % 4) * 128:(st % 4 + 1) * 128],
                    wsl("wv", d, 0, 384),
                    start=(d == 0),
                    stop=(d == ND - 1),
                )
            vt = p_v.tile([128, 384], F16, tag="v")
            nc.vector.tensor_copy(vt[:], ps[:])
            vts.append(vt)
        vts3 = [None] * KT

        def vh3_step(st):
            ps = ps_pv.tile([128, 128], F32, tag="pv", name="vps3")
            for d in range(ND):
                nc.tensor.matmul(
                    ps[:],
                    xsl(d, st // 4)[:, (st % 4) * 128:(st % 4 + 1) * 128],
                    wsl("wv", d, 384, 512),
                    start=(d == 0),
                    stop=(d == ND - 1),
                )
            vt = p_v.tile([128, 128], F16, tag="v3")
            nc.vector.tensor_copy(vt[:], ps[:])
            vts3[st] = vt

        def proj_steps(h):
            """Yield once per PE-chunk of head h's q/k projections."""
            qt = p_qt.tile([128, S], F16, tag="qt")
            kt = p_qt.tile([128, S], F16, tag="kt")
            for dst, wname in ((qt, "wq"), (kt, "wk")):
                for sb in range(NSB):
                    ps = ps_pj.tile([128, 512], F32, tag="proj")
                    for d in range(ND):
                        nc.tensor.matmul(
                            ps[:],
                            wsl(wname, d, h * 128, (h + 1) * 128),
                            xsl(d, sb),
                            start=(d == 0),
                            stop=(d == ND - 1),
                        )
                        if d % 2 == 1:
                            yield None
                    nc.vector.tensor_copy(dst[:, sb * 512:(sb + 1) * 512], ps[:])
            while True:
                yield (qt, kt)

        def rc_chain(sms, deng, ceng=None):
            """sms: two [8,512] PSUM tiles holding the replicated key-sums.
            Returns the broadcast f16 reciprocal [128,1024]; DMAs issue on
            `deng` and the PSUM->SBUF copies on `ceng` so the two tail
            chains can run on separate engines."""
            sm_sb = p_rc.tile([8, 1024], F16, tag="sm_sb")
            if ceng is None:
                nc.vector.tensor_copy(sm_sb[:, :512], sms[0][:])
                nc.vector.tensor_copy(sm_sb[:, 512:], sms[1][:])
            else:
                ceng.copy(sm_sb[:, :512], sms[0][:])
                ceng.copy(sm_sb[:, 512:], sms[1][:])
            rc_in = p_rc.tile([128, 8], F16, tag="rc_in")
            deng.dma_start(rc_in[:], sm_sb[0:1, :], single_packet=True)
            rc_out = p_rc.tile([128, 8], F16, tag="rc_out")
            with nc.allow_low_precision(reason="f16 softmax denominator is ample for the 2e-2 gate"):
                nc.vector.reciprocal(rc_out[:], rc_in[:])
            r2dram = p_dram.tile([1, 1024], F16, tag="r2dram")
            deng.dma_start(
                r2dram[:].rearrange("a (p c) -> (a p) c", p=128), rc_out[:],
                single_packet=True,
            )
            rbc = p_rbc.tile([128, 1024], F16, tag="rbc")
            deng.dma_start(rbc[:], r2dram[0:1, :].to_broadcast((128, 1024)))
            return rbc

        def fold(acc, ets, k):
            # eager fold: acc = et0 + et1, then acc += etk
            if k == 1:
                nc.vector.tensor_add(acc[:], ets[0][:], ets[1][:])
            elif k > 1:
                nc.vector.tensor_add(acc[:], acc[:], ets[k][:])

        def qk_mm(st_ps, qt, kt, q0, k):
            for hf in range(2):
                nc.tensor.matmul(
                    st_ps[:, hf * 512:(hf + 1) * 512],
                    kt[:, k * 128:(k + 1) * 128],
                    qt[:, q0 + hf * 512:q0 + (hf + 1) * 512],
                    start=True,
                    stop=True,
                )

        def attention_block(h, qb, qt, kt, next_proj, tail_hook=None):
            pv = ps_pv.tile([128, 1024], F32, tag="pv")
            q0 = qb * 1024
            ets = {}
            acc = p_acc.tile([128, 1024], F16, tag="acc")

            def qk_step(k):
                st_ps = ps_mm.tile([128, 1024], F32, tag="sT")
                qk_mm(st_ps, qt, kt, q0, k)
                et = p_exp.tile([128, 1024], F16, tag="exp")
                nc.scalar.activation(
                    et[:], st_ps[:], mybir.ActivationFunctionType.Exp,
                    scale=SCALE,
                )
                ets[k] = et
                fold(acc, ets, k)

            def pv_step(k):
                et = ets.pop(k)
                for hf in range(2):
                    sl = slice(hf * 512, (hf + 1) * 512)
                    nc.tensor.matmul(
                        pv[:, sl],
                        vts[k][:, h * 128:(h + 1) * 128],
                        et[:, sl],
                        start=(k == 0),
                        stop=(k == KT - 1),
                    )

            def sum_chain():
                sms = []
                for hf in range(2):
                    sm = ps_pj.tile([8, 512], F32, tag="proj")
                    nc.tensor.matmul(
                        sm[:], ones[:], acc[:, hf * 512:(hf + 1) * 512],
                        start=True, stop=True,
                    )
                    sms.append(sm)
                return rc_chain(sms, nc.sync)

            LAG = 4
            for k in range(KT):
                qk_step(k)
                if k >= LAG:
                    pv_step(k - LAG)
                if next_proj is not None:
                    next(next_proj)
            if tail_hook is not None:
                tail_hook()
            for k in range(KT - LAG, KT - 2):
                pv_step(k)
            rbc = sum_chain()
            pv_step(KT - 2)
            pv_step(KT - 1)

            # free the pv PSUM bank immediately: the multiply waits on the
            # (DMA-latency) reciprocal chain, the copy does not.
            pv_sb = p_out.tile([128, 1024], F32, tag="pvsb")
            nc.vector.tensor_copy(pv_sb[:], pv[:])
            ob = p_out.tile([128, 1024], F16, tag="o")
            nc.gpsimd.tensor_mul(ob[:], pv_sb[:], rbc[:])
            nc.sync.dma_start(
                out_ap[h * 128:(h + 1) * 128, qb * 1024:(qb + 1) * 1024], ob[:]
            )

        def sb_make(qt, kt):
            """Shared state for the last head's superblock; created early so
            its first qk pair can be emitted inside the previous block's
            tail (keeps the ACT engine streaming across the boundary)."""
            ets0, ets1 = {}, {}
            acc0 = p_acc.tile([128, 1024], F16, tag="acc", name="acc0")
            acc1 = p_acc.tile([128, 1024], F16, tag="acc", name="acc1")

            def qk_step(qb, k):
                ets, acc = (ets0, acc0) if qb == 0 else (ets1, acc1)
                st_ps = ps_mm.tile([128, 1024], F32, tag="sT", name="sbst")
                qk_mm(st_ps, qt, kt, qb * 1024, k)
                et = p_exp.tile([128, 1024], F16, tag="exp", name="sbet")
                nc.scalar.activation(
                    et[:], st_ps[:], mybir.ActivationFunctionType.Exp,
                    scale=SCALE,
                )
                ets[k] = et
                fold(acc, ets, k)
            return {"ets0": ets0, "ets1": ets1, "acc0": acc0, "acc1": acc1,
                    "qk_step": qk_step}

        def attention_superblock(h, ctx):
            """Both query blocks of the last head, interleaved; qb1's PV runs
            as a trailing burst that covers both softmax-sum chains, so only
            the final multiply + store trail the kernel. qb0's PV accumulates
            in the (idle) projection PSUM pool as two half-tiles."""
            pv0 = [ps_pj.tile([128, 512], F32, tag="proj", name=f"pv0h{hf}")
                   for hf in range(2)]
            pv1 = ps_pv.tile([128, 1024], F32, tag="pv")
            ets0, ets1 = ctx["ets0"], ctx["ets1"]
            acc0, acc1 = ctx["acc0"], ctx["acc1"]
            qk_step = ctx["qk_step"]

            def pv0_step(k):
                et = ets0.pop(k)
                for hf in range(2):
                    sl = slice(hf * 512, (hf + 1) * 512)
                    nc.tensor.matmul(
                        pv0[hf][:],
                        vts3[k][:],
                        et[:, sl],
                        start=(k == 0),
                        stop=(k == KT - 1),
                    )

            def pv1_step(k):
                et = ets1[k]
                for hf in range(2):
                    sl = slice(hf * 512, (hf + 1) * 512)
                    nc.tensor.matmul(
                        pv1[:, sl],
                        vts3[k][:],
                        et[:, sl],
                        start=(k == 0),
                        stop=(k == KT - 1),
                    )

            def rc_flat(acc, deng, ceng):
                """key-sums of acc -> f16 reciprocal laid [1,1024] in SBUF."""
                sm = ps_mm.tile([8, 1024], F32, tag="sT", name="sm")
                for hf in range(2):
                    nc.tensor.matmul(
                        sm[:, hf * 512:(hf + 1) * 512], ones[:],
                        acc[:, hf * 512:(hf + 1) * 512],
                        start=True, stop=True,
                    )
                sm_sb = p_rc.tile([8, 1024], F16, tag="sm_sb")
                if ceng is None:
                    nc.vector.tensor_copy(sm_sb[:, :512], sm[:, :512])
                    nc.vector.tensor_copy(sm_sb[:, 512:], sm[:, 512:])
                else:
                    # split across ACT+DVE: this copy pair gates the last
                    # reciprocal chain of the kernel
                    ceng.copy(sm_sb[:, :512], sm[:, :512])
                    nc.vector.tensor_copy(sm_sb[:, 512:], sm[:, 512:])
                rc_in = p_rc.tile([128, 8], F16, tag="rc_in")
                deng.dma_start(rc_in[:], sm_sb[0:1, :], single_packet=True)
                rc_out = p_rc.tile([128, 8], F16, tag="rc_out")
                with nc.allow_low_precision(reason="f16 softmax denominator is ample for the 2e-2 gate"):
                    nc.vector.reciprocal(rc_out[:], rc_in[:])
                rcf = p_rc.tile([1, 1024], F16, tag="rcf")
                deng.dma_start(rcf[:], rc_out[:], single_packet=True)
                return rcf

            def rbc_mm(rcf):
                """broadcast the [1,1024] reciprocal to [128,1024] via PE."""
                rbc = ps_mm.tile([128, 1024], F32, tag="sT", name="rbc")
                for hf in range(2):
                    nc.tensor.matmul(
                        rbc[:, hf * 512:(hf + 1) * 512], ones1[:],
                        rcf[0:1, hf * 512:(hf + 1) * 512],
                        start=True, stop=True,
                    )
                return rbc

            vh3_step(0)
            qk_step(0, 1)
            vh3_step(1)
            qk_step(1, 1)
            for k in range(2, KT - 1):
                pv0_step(k - 2)
                qk_step(0, k)
                vh3_step(k)
                qk_step(1, k)
            qk_step(0, KT - 1)
            qk_step(1, KT - 1)
            vh3_step(KT - 1)
            pv0_step(KT - 3)
            pv0_step(KT - 2)
            pv0_step(KT - 1)
            pv1_step(0)
            pv1_step(1)
            rcf0 = rc_flat(acc0, nc.sync, None)
            pv1_step(2)
            pv1_step(3)
            # pv0 -> SBUF as soon as it stops (mid pv1-burst): the final
            # multiplies need SBUF operands (one-PSUM-operand rule), and
            # these copies must not queue on DVE behind the kernel tail.
            pv0_sb = p_out.tile([128, 1024], F32, tag="pvsb", name="pv0sb")
            for hf in range(2):
                nc.vector.tensor_copy(pv0_sb[:, hf * 512:(hf + 1) * 512], pv0[hf][:])
            rcf1 = rc_flat(acc1, nc.scalar, nc.scalar)
            for k in range(4, KT):
                pv1_step(k)

            pv1_sb = p_out.tile([128, 1024], F32, tag="pvsb", name="pv1sb")
            nc.vector.tensor_copy(pv1_sb[:], pv1[:])

            rbc0 = rbc_mm(rcf0)
            rbc1 = rbc_mm(rcf1)
            ob0 = p_out.tile([128, 1024], F16, tag="o")
            nc.vector.tensor_mul(ob0[:, :512], pv0_sb[:, :512], rbc0[:, :512])
            nc.vector.tensor_mul(ob0[:, 512:], pv0_sb[:, 512:], rbc0[:, 512:])
            nc.sync.dma_start(out_ap[h * 128:(h + 1) * 128, 0:512], ob0[:, :512])
            nc.sync.dma_start(out_ap[h * 128:(h + 1) * 128, 512:1024], ob0[:, 512:])
            ob1 = p_out.tile([128, 1024], F16, tag="o")
            nc.vector.tensor_mul(ob1[:, :512], pv1_sb[:, :512], rbc1[:, :512])
            nc.vector.tensor_mul(ob1[:, 512:], pv1_sb[:, 512:], rbc1[:, 512:])
            nc.scalar.dma_start(
                out_ap[h * 128:(h + 1) * 128, 1024:1536], ob1[:, :512])
            nc.scalar.dma_start(
                out_ap[h * 128:(h + 1) * 128, 1536:2048], ob1[:, 512:])

        # head 0's projections run serially (nothing to hide them under);
        # heads 1..3 project inside the previous head's attention loop.
        gen = proj_steps(0)
        res = None
        while not isinstance(res, tuple):
            res = next(gen)
        qt, kt = res
        sb_state = {}

        def sb_warm(gen):
            res = None
            while not isinstance(res, tuple):
                res = next(gen)
            qt3, kt3 = res
            ctx = sb_make(qt3, kt3)
            ctx["qk_step"](0, 0)
            ctx["qk_step"](1, 0)
            sb_state["ctx"] = ctx
            sb_state["qtkt"] = (qt3, kt3)

        for h in range(HPC):
            nxt = proj_steps(h + 1) if h + 1 < HPC else None
            if nxt is None:
                attention_superblock(h, sb_state["ctx"])
            else:
                for qb in range(2):
                    hook = None
                    if h == HPC - 2 and qb == 1:
                        hook = lambda: sb_warm(nxt)
                    attention_block(h, qb, qt, kt, nxt, tail_hook=hook)
            if nxt is not None:
                if h == HPC - 2:
                    qt, kt = sb_state["qtkt"]
                else:
                    res = None
                    while not isinstance(res, tuple):
                        res = next(nxt)
                    qt, kt = res


def _build():
    nc = bacc.Bacc(
        "TRN2",
        target_bir_lowering=False,
        debug=False,
        enable_asserts=False,
        num_devices=N_CORES,
    )
    xt_ap = nc.dram_tensor("xt", [D, S], F16, kind="ExternalInput").ap()
    wq_ap = nc.dram_tensor("wq", [D, DHG], F16, kind="ExternalInput").ap()
    wk_ap = nc.dram_tensor("wk", [D, DHG], F16, kind="ExternalInput").ap()
    wv_ap = nc.dram_tensor("wv", [D, DHG], F16, kind="ExternalInput").ap()
    out_ap = nc.dram_tensor("out", [DHG, S], F16, kind="ExternalOutput").ap()
    with tile.TileContext(nc) as tc:
        _emit(tc, nc, xt_ap, wq_ap, wk_ap, wv_ap, out_ap)
    nc.compile()
    return nc


def _shard_inputs(inputs):
    x = np.ascontiguousarray(np.asarray(inputs["input_embeddings"], dtype=np.float32))
    wq = np.asarray(inputs["w_query"], dtype=np.float32)
    wk = np.asarray(inputs["w_key"], dtype=np.float32)
    wv = np.asarray(inputs["w_value"], dtype=np.float32)

    def gather(w, g):
        # head h occupies the strided cols d = hd*8 + h; regroup head-major
        w4 = w.reshape(D, DH, H)[:, :, g * HPC:(g + 1) * HPC]   # (D, hd, hl)
        return np.ascontiguousarray(w4.transpose(0, 2, 1).reshape(D, DHG).astype(np.float16))

    in_maps = []
    for c in range(N_CORES):
        b, g = divmod(c, 2)
        in_maps.append(
            {
                "xt": np.ascontiguousarray(x[b].T.astype(np.float16)),
                "wq": gather(wq, g),
                "wk": gather(wk, g),
                "wv": gather(wv, g),
            }
        )
    return in_maps


def kernel(**inputs):
    nc = _CACHE.get("nc")
    if nc is None:
        nc = _CACHE["nc"] = _build()
    in_maps = _shard_inputs(inputs)
    res = run_bass_kernel_spmd(
        nc, in_maps, core_ids=list(range(N_CORES)), trace=TRACE
    )
    _CACHE["last_result"] = res
    out = np.empty((B, S, DH, H), dtype=np.float32)
    for c in range(N_CORES):
        b, g = divmod(c, 2)
        o = res.results[c]["out"].astype(np.float32).reshape(HPC, DH, S)  # (hl, hd, s)
        out[b, :, :, g * HPC:(g + 1) * HPC] = o.transpose(2, 1, 0)
    return out.reshape(B, S, D)
